# revision 13
# baseline (speedup 1.0000x reference)
"""Trainium2 Bass kernel for nn_Attention1 — v4.

Per batch b: out[b] = w @ x with w[k] = sum_q softmax(q x^T/16)[q, k]
(query-summed attention; only softmax column sums are needed).
Data parallel: one batch element per NeuronCore.

Structure:
  Phase A (stage-batched, x-path first so scores can start early):
    x: load f32 -> fp16 cast (DVE 2x) -> DRAM bounce -> XBAR transpose
       -> fp8 cast (GpSimd)  => xT8 [d%128, d//128, s]
    q: mask load; q16 = fp16(x*mask) on GpSimd; diag via q16^2 (DVE TT 2x +
       reduce); exp biases; bounce/transpose/fp8 cast as for x.
  Phase B (32 query stripes of 128):
    scores: fp8 DoubleRow matmuls, K=256 in one pass, N=512 per bank.
    exp: ScalarE activation (bias/scale fused, accum_out row sums) on part
    of each stripe; VectorE fast exp2 (int-bit trick, one fused
    tensor_scalar) + convert/accumulate pass on the rest.
    column sums: M=1 matmuls with r_q = fp16(1/Z_q) stationary, accumulated
    in 2 persistent PSUM banks across all 32 stripes (4 col-groups/bank).
  Tail: evacuate w, transpose into partitions with K=1 matmuls,
    out = w @ x in fp16, DMA out.

Known stack pitfalls honored here: tensor_tensor_reduce aborts at runtime
(use TT+reduce or activation accum); SBUF<->SBUF DMA deadlocks against
XBAR transposes (casts are compute-engine copies); CCE mult-during-DMA
unsupported.
"""

import os
import sys

import numpy as np

_TRN_REPO = "/opt/trn_rl_repo"
if os.path.isdir(_TRN_REPO) and _TRN_REPO not in sys.path:
    sys.path.insert(0, _TRN_REPO)

import concourse.bass as bass
import concourse.mybir as mybir
import concourse.tile as tile
from concourse import bacc
from concourse.bass_utils import run_bass_kernel_spmd

F32 = mybir.dt.float32
F16 = mybir.dt.float16
FP8 = mybir.dt.float8e4
I32 = mybir.dt.int32
OP = mybir.AluOpType
DR = mybir.MatmulPerfMode.DoubleRow

B = 8
S = 4096
D = 256
P = 128

NBLK = S // 512
NST = S // P

E_SHIFT = float(8 * np.log(2.0))
SCALE = 1.0 / 16.0

C_MAGIC = 361007.0
K2 = float((2.0**23) * np.log2(np.e) / 16.0)
CB = float((127.0 * 2.0**23 - C_MAGIC) * 16.0 / ((2.0**23) * np.log2(np.e)))

# per-stripe k tiles and their exp consumers: (k0, kn, [(who, c0, c1), ...])
# "A" = ScalarE activation exp, "V" = VectorE fast exp. Ranges relative to
# tile start. zi = index of the per-stripe row-sum partial each consumer owns.
K_TILES = [
    (0, 1536, [("A", 0, 1536, 0)]),
    (1536, 1536, [("V", 0, 1536, 2)]),
    (3072, 1024, [("V", 0, 512, 3), ("A", 512, 1024, 1)]),
]
NZ = 4


def build_kernel(finalize: bool = True) -> bass.Bass:
    nc = bacc.Bacc(None)

    x_in = nc.declare_dram_parameter("x", [S, D], F32, isOutput=False)
    m_in = nc.declare_dram_parameter("mask", [S, D], F32, isOutput=False)
    out_ext = nc.declare_dram_parameter("out", [1, D], F32, isOutput=True)

    x_in_t = x_in.rearrange("(a p) d -> p a d", p=P)
    m_in_t = m_in.rearrange("(a p) d -> p a d", p=P)

    with tile.TileContext(nc) as tc:
        with (
            tc.tile_pool(name="dram", bufs=1, space="DRAM") as dramp,
            tc.tile_pool(name="resident", bufs=1) as res,
            tc.tile_pool(name="mstage", bufs=3) as mstage,
            tc.tile_pool(name="qstage", bufs=3) as qstage,
            tc.tile_pool(name="tstage", bufs=4) as tstage,
            tc.tile_pool(name="etile", bufs=8) as ep,
            tc.tile_pool(name="e32", bufs=3) as e32p,
            tc.tile_pool(name="ps_scores", bufs=2, space="PSUM") as pss,
            tc.tile_pool(name="ps_w", bufs=2, space="PSUM") as psw,
        ):
            q16d = dramp.tile([S, D], F16)
            x16d = dramp.tile([S, D], F16)

            xf = res.tile([P, NST, D], F32)
            x16 = res.tile([P, NST, D], F16)
            qT8 = res.tile([P, 2, S], FP8)
            xT8 = res.tile([P, 2, S], FP8)
            bias_all = res.tile([P, NST], F32)
            bias2_all = res.tile([P, NST], F32)
            d2 = res.tile([P, NST], F32)
            zP = [res.tile([P, NST], F32, name=f"zp{i}") for i in range(NZ)]
            zsum = res.tile([P, NST], F32)
            rr = res.tile([P, NST], F32)
            r16 = res.tile([P, NST], F16)
            w16 = res.tile([1, S], F16)
            wtot_sb = res.tile([P, NST], F16)
            ones16 = res.tile([1, 1], F16)
            out_sb = res.tile([1, D], F32)

            nc.vector.memset(ones16[:], 1.0)

            wb = [
                psw.tile([P, 512], F32, tag="w", name=f"wb{i}") for i in range(2)
            ]

            # ---- Phase A ----
            # x-path, stage-batched so the 8 blocks' DMAs pipeline
            for blk in range(NBLK):
                a0 = blk * 4
                nc.scalar.dma_start(
                    xf[:, a0 : a0 + 4, :], x_in_t[:, a0 : a0 + 4, :]
                )
            for blk in range(NBLK):
                a0 = blk * 4
                nc.vector.tensor_copy(
                    x16[:, a0 : a0 + 4, :], xf[:, a0 : a0 + 4, :]
                )
                nc.scalar.dma_start(
                    x16d.rearrange("(a p) d -> p a d", p=P)[:, a0 : a0 + 4, :],
                    x16[:, a0 : a0 + 4, :],
                )
            for blk in range(NBLK):
                rows = slice(blk * 512, (blk + 1) * 512)
                xT16b = tstage.tile([P, 2, 512], F16, tag="xT16b")
                for dh in range(2):
                    nc.sync.dma_start(
                        xT16b[:, dh, :],
                        x16d[rows, dh * P : (dh + 1) * P],
                        transpose=True,
                    )
                nc.gpsimd.tensor_copy(xT8[:, :, rows], xT16b[:])

            # q-path, per block
            for blk in range(NBLK):
                a0 = blk * 4
                rows = slice(blk * 512, (blk + 1) * 512)
                mk = mstage.tile([P, 4, D], F32, tag="mk")
                nc.sync.dma_start(mk[:], m_in_t[:, a0 : a0 + 4, :])
                q16 = qstage.tile([P, 4, D], F16, tag="q16")
                nc.gpsimd.tensor_tensor(
                    q16[:], xf[:, a0 : a0 + 4, :], mk[:], OP.mult
                )
                # d2 = sum_d q16^2 = 2*diag (mask^2 = 2*mask)
                q2 = qstage.tile([P, 4, D], F16, tag="q2")
                nc.vector.tensor_tensor(q2[:], q16[:], q16[:], OP.mult)
                nc.vector.tensor_reduce(
                    d2[:, a0 : a0 + 4], q2[:], mybir.AxisListType.X, OP.add
                )
                nc.vector.tensor_scalar(
                    bias_all[:, a0 : a0 + 4], d2[:, a0 : a0 + 4],
                    -1.0 / 32.0, E_SHIFT, OP.mult, OP.add,
                )
                nc.vector.tensor_scalar(
                    bias2_all[:, a0 : a0 + 4], d2[:, a0 : a0 + 4],
                    -0.5, 16.0 * E_SHIFT + CB, OP.mult, OP.add,
                )
                nc.scalar.dma_start(
                    q16d.rearrange("(a p) d -> p a d", p=P)[:, a0 : a0 + 4, :],
                    q16[:],
                )
                qT16b = tstage.tile([P, 2, 512], F16, tag="qT16b")
                for dh in range(2):
                    nc.sync.dma_start(
                        qT16b[:, dh, :],
                        q16d[rows, dh * P : (dh + 1) * P],
                        transpose=True,
                    )
                nc.gpsimd.tensor_copy(qT8[:, :, rows], qT16b[:])

            # ---- Phase B ----
            def emit_colsum(g, ets):
                for j in range(4):
                    qs = 4 * g + j
                    for c in range(8):
                        nc.tensor.matmul(
                            wb[c // 4][32 * (c % 4) : 32 * (c % 4) + 1, :],
                            lhsT=r16[:, qs : qs + 1],
                            rhs=ets[j][:, c * 512 : (c + 1) * 512],
                            start=(qs == 0),
                            stop=(qs == 31),
                            tile_position=(0, 32 * (c % 4)),
                            skip_group_check=True,
                        )

            prev = None
            for g in range(8):
                ets = []
                for j in range(4):
                    qs = 4 * g + j
                    et = ep.tile([P, S], F16, tag="e")
                    ets.append(et)
                    for k0, kn, consumers in K_TILES:
                        ps = pss.tile([P, 1536], F32, tag="s")
                        for n in range(kn // 512):
                            nc.tensor.matmul(
                                ps[:, n * 512 : (n + 1) * 512],
                                lhsT=qT8[:, :, qs * P : (qs + 1) * P],
                                rhs=xT8[:, :, k0 + n * 512 : k0 + (n + 1) * 512],
                                start=True,
                                stop=True,
                                perf_mode=DR,
                            )
                        for who, c0, c1, zi in consumers:
                            if who == "A":
                                nc.scalar.activation(
                                    out=et[:, k0 + c0 : k0 + c1],
                                    in_=ps[:, c0:c1],
                                    func=mybir.ActivationFunctionType.Exp,
                                    bias=bias_all[:, qs : qs + 1],
                                    scale=SCALE,
                                    accum_out=zP[zi][:, qs : qs + 1],
                                )
                            else:
                                e32 = e32p.tile([P, 1536], F32, tag="e32")
                                nc.vector.tensor_scalar(
                                    e32.bitcast(I32)[:, : c1 - c0],
                                    ps[:, c0:c1],
                                    bias2_all[:, qs : qs + 1],
                                    K2,
                                    OP.add,
                                    OP.mult,
                                )
                                nc.vector.tensor_scalar(
                                    et[:, k0 + c0 : k0 + c1],
                                    e32[:, : c1 - c0],
                                    1.0,
                                    None,
                                    OP.mult,
                                    OP.add,
                                    accum_out=zP[zi][:, qs : qs + 1],
                                )
                    if j == 0 and prev is not None:
                        emit_colsum(g - 1, prev)
                        prev = None
                sl = slice(4 * g, 4 * g + 4)
                nc.vector.tensor_tensor(zsum[:, sl], zP[0][:, sl], zP[1][:, sl], OP.add)
                nc.vector.tensor_tensor(zsum[:, sl], zsum[:, sl], zP[2][:, sl], OP.add)
                nc.vector.tensor_tensor(zsum[:, sl], zsum[:, sl], zP[3][:, sl], OP.add)
                nc.vector.reciprocal(rr[:, sl], zsum[:, sl])
                nc.vector.tensor_copy(r16[:, sl], rr[:, sl])
                prev = ets
            emit_colsum(7, prev)

            # ---- Tail ----
            for c in range(8):
                src = wb[c // 4][32 * (c % 4) : 32 * (c % 4) + 1, :]
                dst = w16[:, c * 512 : (c + 1) * 512]
                if c % 2 == 0:
                    nc.vector.tensor_copy(dst, src)
                else:
                    nc.scalar.copy(dst, src)

            wtotP = psw.tile([P, NST], F32, tag="w")
            for cc in range(NST):
                nc.tensor.matmul(
                    wtotP[:, cc : cc + 1],
                    lhsT=w16[:, cc * P : (cc + 1) * P],
                    rhs=ones16[:],
                    start=True,
                    stop=True,
                )
            nc.vector.tensor_copy(wtot_sb[:], wtotP[:])

            po = psw.tile([1, D], F32, tag="w")
            for cc in range(NST):
                nc.tensor.matmul(
                    po[:],
                    lhsT=wtot_sb[:, cc : cc + 1],
                    rhs=x16[:, cc, :],
                    start=(cc == 0),
                    stop=(cc == NST - 1),
                )
            nc.scalar.copy(out_sb[:], po[:])
            nc.sync.dma_start(out_ext[:, :], out_sb[:])

    if finalize:
        nc.finalize()
    return nc


def _run(x: np.ndarray, drop_mask: np.ndarray, trace: bool = False, nc=None):
    if nc is None:
        nc = build_kernel()
    in_maps = [{"x": x[b], "mask": drop_mask[b]} for b in range(B)]
    res = run_bass_kernel_spmd(nc, in_maps, list(range(B)), trace=trace)
    out = np.stack([res.results[b]["out"].reshape(D) for b in range(B)])
    return out.astype(np.float32), res


def kernel(**inputs: np.ndarray) -> np.ndarray:
    x = np.ascontiguousarray(inputs["x"], dtype=np.float32)
    drop_mask = np.ascontiguousarray(inputs["drop_mask"], dtype=np.float32)
    assert x.shape == (B, S, D) and drop_mask.shape == (B, S, D)
    out, _ = _run(x, drop_mask)
    return out


def profile(**inputs: np.ndarray):
    x = np.ascontiguousarray(inputs["x"], dtype=np.float32)
    drop_mask = np.ascontiguousarray(inputs["drop_mask"], dtype=np.float32)
    out, res = _run(x, drop_mask, trace=True)
    return res.exec_time_ns


if __name__ == "__main__":
    rng = np.random.default_rng(0)
    x = rng.standard_normal((B, S, D)).astype(np.float32)
    m = (rng.random((B, S, D)) < 0.5).astype(np.float32) * 2.0
    out = kernel(x=x, drop_mask=m)
    print(out.shape, out.dtype)


# revision 14
# speedup vs baseline: 1.3550x; 1.3550x over previous
"""Trainium2 Bass kernel for nn_Attention1 — v4.

Per batch b: out[b] = w @ x with w[k] = sum_q softmax(q x^T/16)[q, k]
(query-summed attention; only softmax column sums are needed).
Data parallel: one batch element per NeuronCore.

Structure:
  Phase A (stage-batched, x-path first so scores can start early):
    x: load f32 -> fp16 cast (DVE 2x) -> DRAM bounce -> XBAR transpose
       -> fp8 cast (GpSimd)  => xT8 [d%128, d//128, s]
    q: mask load; q16 = fp16(x*mask) on GpSimd; diag via q16^2 (DVE TT 2x +
       reduce); exp biases; bounce/transpose/fp8 cast as for x.
  Phase B (32 query stripes of 128):
    scores: fp8 DoubleRow matmuls, K=256 in one pass, N=512 per bank.
    exp: ScalarE activation (bias/scale fused, accum_out row sums) on part
    of each stripe; VectorE fast exp2 (int-bit trick, one fused
    tensor_scalar) + convert/accumulate pass on the rest.
    column sums: M=1 matmuls with r_q = fp16(1/Z_q) stationary, accumulated
    in 2 persistent PSUM banks across all 32 stripes (4 col-groups/bank).
  Tail: evacuate w, transpose into partitions with K=1 matmuls,
    out = w @ x in fp16, DMA out.

Known stack pitfalls honored here: tensor_tensor_reduce aborts at runtime
(use TT+reduce or activation accum); SBUF<->SBUF DMA deadlocks against
XBAR transposes (casts are compute-engine copies); CCE mult-during-DMA
unsupported.
"""

import os
import sys

import numpy as np

_TRN_REPO = "/opt/trn_rl_repo"
if os.path.isdir(_TRN_REPO) and _TRN_REPO not in sys.path:
    sys.path.insert(0, _TRN_REPO)

import concourse.bass as bass
import concourse.mybir as mybir
import concourse.tile as tile
from concourse import bacc
from concourse.bass_utils import run_bass_kernel_spmd

F32 = mybir.dt.float32
F16 = mybir.dt.float16
FP8 = mybir.dt.float8e4
I32 = mybir.dt.int32
OP = mybir.AluOpType
DR = mybir.MatmulPerfMode.DoubleRow

B = 8
S = 4096
D = 256
P = 128

NBLK = S // 512
NST = S // P

E_SHIFT = float(8 * np.log(2.0))
SCALE = 1.0 / 16.0

C_MAGIC = 361007.0
K2 = float((2.0**23) * np.log2(np.e) / 16.0)
CB = float((127.0 * 2.0**23 - C_MAGIC) * 16.0 / ((2.0**23) * np.log2(np.e)))

# per-stripe k tiles and their exp consumers: (k0, kn, [(who, c0, c1), ...])
# "A" = ScalarE activation exp, "V" = VectorE fast exp. Ranges relative to
# tile start. zi = index of the per-stripe row-sum partial each consumer owns.
K_TILES = [
    (0, 1536, [("A", 0, 1536, 0)]),
    (1536, 1536, [("V", 0, 1536, 2)]),
    (3072, 1024, [("A", 0, 1024, 1)]),
]
NZ = 3


def build_kernel(finalize: bool = True) -> bass.Bass:
    nc = bacc.Bacc(None)

    x_in = nc.declare_dram_parameter("x", [S, D], F32, isOutput=False)
    m_in = nc.declare_dram_parameter("mask", [S, D], F32, isOutput=False)
    out_ext = nc.declare_dram_parameter("out", [1, D], F32, isOutput=True)

    x_in_t = x_in.rearrange("(a p) d -> p a d", p=P)
    m_in_t = m_in.rearrange("(a p) d -> p a d", p=P)

    with tile.TileContext(nc) as tc:
        with (
            tc.tile_pool(name="dram", bufs=1, space="DRAM") as dramp,
            tc.tile_pool(name="resident", bufs=1) as res,
            tc.tile_pool(name="mstage", bufs=3) as mstage,
            tc.tile_pool(name="qstage", bufs=3) as qstage,
            tc.tile_pool(name="tstage", bufs=4) as tstage,
            tc.tile_pool(name="etile", bufs=8) as ep,
            tc.tile_pool(name="e32", bufs=3) as e32p,
            tc.tile_pool(name="ps_scores", bufs=2, space="PSUM") as pss,
            tc.tile_pool(name="ps_w", bufs=2, space="PSUM") as psw,
        ):
            q16d = dramp.tile([S, D], F16)
            x16d = dramp.tile([S, D], F16)

            xf = res.tile([P, NST, D], F32)
            x16 = res.tile([P, NST, D], F16)
            qT8 = res.tile([P, 2, S], FP8)
            xT8 = res.tile([P, 2, S], FP8)
            bias_all = res.tile([P, NST], F32)
            bias2_all = res.tile([P, NST], F32)
            d2 = res.tile([P, NST], F32)
            zP = [res.tile([P, NST], F32, name=f"zp{i}") for i in range(NZ)]
            zsum = res.tile([P, NST], F32)
            rr = res.tile([P, NST], F32)
            r16 = res.tile([P, NST], F16)
            w16 = res.tile([1, S], F16)
            wtot_sb = res.tile([P, NST], F16)
            ones16 = res.tile([1, 1], F16)
            out_sb = res.tile([1, D], F32)

            nc.vector.memset(ones16[:], 1.0)

            wb = [
                psw.tile([P, 512], F32, tag="w", name=f"wb{i}") for i in range(2)
            ]

            # ---- Phase A ----
            # x-path, stage-batched so the 8 blocks' DMAs pipeline
            for blk in range(NBLK):
                a0 = blk * 4
                nc.scalar.dma_start(
                    xf[:, a0 : a0 + 4, :], x_in_t[:, a0 : a0 + 4, :]
                )
            for blk in range(NBLK):
                a0 = blk * 4
                nc.vector.tensor_copy(
                    x16[:, a0 : a0 + 4, :], xf[:, a0 : a0 + 4, :]
                )
                nc.scalar.dma_start(
                    x16d.rearrange("(a p) d -> p a d", p=P)[:, a0 : a0 + 4, :],
                    x16[:, a0 : a0 + 4, :],
                )
            for blk in range(NBLK):
                rows = slice(blk * 512, (blk + 1) * 512)
                xT16b = tstage.tile([P, 2, 512], F16, tag="xT16b")
                for dh in range(2):
                    nc.sync.dma_start(
                        xT16b[:, dh, :],
                        x16d[rows, dh * P : (dh + 1) * P],
                        transpose=True,
                    )
                nc.scalar.copy(xT8[:, :, rows], xT16b[:])

            # q-path, per block
            for blk in range(NBLK):
                a0 = blk * 4
                rows = slice(blk * 512, (blk + 1) * 512)
                mk = mstage.tile([P, 4, D], F32, tag="mk")
                nc.sync.dma_start(mk[:], m_in_t[:, a0 : a0 + 4, :])
                q16 = qstage.tile([P, 4, D], F16, tag="q16")
                nc.gpsimd.tensor_tensor(
                    q16[:], xf[:, a0 : a0 + 4, :], mk[:], OP.mult
                )
                # d2 = sum_d q16^2 = 2*diag (mask^2 = 2*mask)
                q2 = qstage.tile([P, 4, D], F16, tag="q2")
                nc.vector.tensor_tensor(q2[:], q16[:], q16[:], OP.mult)
                nc.vector.tensor_reduce(
                    d2[:, a0 : a0 + 4], q2[:], mybir.AxisListType.X, OP.add
                )
                nc.vector.tensor_scalar(
                    bias_all[:, a0 : a0 + 4], d2[:, a0 : a0 + 4],
                    -1.0 / 32.0, E_SHIFT, OP.mult, OP.add,
                )
                nc.vector.tensor_scalar(
                    bias2_all[:, a0 : a0 + 4], d2[:, a0 : a0 + 4],
                    -0.5, 16.0 * E_SHIFT + CB, OP.mult, OP.add,
                )
                nc.scalar.dma_start(
                    q16d.rearrange("(a p) d -> p a d", p=P)[:, a0 : a0 + 4, :],
                    q16[:],
                )
                qT16b = tstage.tile([P, 2, 512], F16, tag="qT16b")
                for dh in range(2):
                    nc.sync.dma_start(
                        qT16b[:, dh, :],
                        q16d[rows, dh * P : (dh + 1) * P],
                        transpose=True,
                    )
                nc.scalar.copy(qT8[:, :, rows], qT16b[:])

            # ---- Phase B ----
            def emit_colsum(g, ets):
                for j in range(4):
                    qs = 4 * g + j
                    for c in range(8):
                        nc.tensor.matmul(
                            wb[c // 4][32 * (c % 4) : 32 * (c % 4) + 1, :],
                            lhsT=r16[:, qs : qs + 1],
                            rhs=ets[j][:, c * 512 : (c + 1) * 512],
                            start=(qs == 0),
                            stop=(qs == 31),
                            tile_position=(0, 32 * (c % 4)),
                            skip_group_check=True,
                        )

            prev = None
            for g in range(8):
                ets = []
                for j in range(4):
                    qs = 4 * g + j
                    et = ep.tile([P, S], F16, tag="e")
                    ets.append(et)
                    for k0, kn, consumers in K_TILES:
                        ps = pss.tile([P, 1536], F32, tag="s")
                        for n in range(kn // 512):
                            nc.tensor.matmul(
                                ps[:, n * 512 : (n + 1) * 512],
                                lhsT=qT8[:, :, qs * P : (qs + 1) * P],
                                rhs=xT8[:, :, k0 + n * 512 : k0 + (n + 1) * 512],
                                start=True,
                                stop=True,
                                perf_mode=DR,
                            )
                        for who, c0, c1, zi in consumers:
                            if who == "A":
                                nc.scalar.activation(
                                    out=et[:, k0 + c0 : k0 + c1],
                                    in_=ps[:, c0:c1],
                                    func=mybir.ActivationFunctionType.Exp,
                                    bias=bias_all[:, qs : qs + 1],
                                    scale=SCALE,
                                    accum_out=zP[zi][:, qs : qs + 1],
                                )
                            else:
                                e32 = e32p.tile([P, 1536], F32, tag="e32")
                                nc.vector.tensor_scalar(
                                    e32.bitcast(I32)[:, : c1 - c0],
                                    ps[:, c0:c1],
                                    bias2_all[:, qs : qs + 1],
                                    K2,
                                    OP.add,
                                    OP.mult,
                                )
                                nc.vector.tensor_scalar(
                                    et[:, k0 + c0 : k0 + c1],
                                    e32[:, : c1 - c0],
                                    1.0,
                                    None,
                                    OP.mult,
                                    OP.add,
                                    accum_out=zP[zi][:, qs : qs + 1],
                                )
                    if j == 0 and prev is not None:
                        emit_colsum(g - 1, prev)
                        prev = None
                sl = slice(4 * g, 4 * g + 4)
                nc.vector.tensor_tensor(zsum[:, sl], zP[0][:, sl], zP[1][:, sl], OP.add)
                nc.vector.tensor_tensor(zsum[:, sl], zsum[:, sl], zP[2][:, sl], OP.add)
                nc.vector.reciprocal(rr[:, sl], zsum[:, sl])
                nc.vector.tensor_copy(r16[:, sl], rr[:, sl])
                prev = ets
            emit_colsum(7, prev)

            # ---- Tail ----
            for c in range(8):
                src = wb[c // 4][32 * (c % 4) : 32 * (c % 4) + 1, :]
                dst = w16[:, c * 512 : (c + 1) * 512]
                if c % 2 == 0:
                    nc.vector.tensor_copy(dst, src)
                else:
                    nc.scalar.copy(dst, src)

            wtotP = psw.tile([P, NST], F32, tag="w")
            for cc in range(NST):
                nc.tensor.matmul(
                    wtotP[:, cc : cc + 1],
                    lhsT=w16[:, cc * P : (cc + 1) * P],
                    rhs=ones16[:],
                    start=True,
                    stop=True,
                )
            nc.vector.tensor_copy(wtot_sb[:], wtotP[:])

            po = psw.tile([1, D], F32, tag="w")
            for cc in range(NST):
                nc.tensor.matmul(
                    po[:],
                    lhsT=wtot_sb[:, cc : cc + 1],
                    rhs=x16[:, cc, :],
                    start=(cc == 0),
                    stop=(cc == NST - 1),
                )
            nc.scalar.copy(out_sb[:], po[:])
            nc.sync.dma_start(out_ext[:, :], out_sb[:])

    if finalize:
        nc.finalize()
    return nc


def _run(x: np.ndarray, drop_mask: np.ndarray, trace: bool = False, nc=None):
    if nc is None:
        nc = build_kernel()
    in_maps = [{"x": x[b], "mask": drop_mask[b]} for b in range(B)]
    res = run_bass_kernel_spmd(nc, in_maps, list(range(B)), trace=trace)
    out = np.stack([res.results[b]["out"].reshape(D) for b in range(B)])
    return out.astype(np.float32), res


def kernel(**inputs: np.ndarray) -> np.ndarray:
    x = np.ascontiguousarray(inputs["x"], dtype=np.float32)
    drop_mask = np.ascontiguousarray(inputs["drop_mask"], dtype=np.float32)
    assert x.shape == (B, S, D) and drop_mask.shape == (B, S, D)
    out, _ = _run(x, drop_mask)
    return out


def profile(**inputs: np.ndarray):
    x = np.ascontiguousarray(inputs["x"], dtype=np.float32)
    drop_mask = np.ascontiguousarray(inputs["drop_mask"], dtype=np.float32)
    out, res = _run(x, drop_mask, trace=True)
    return res.exec_time_ns


if __name__ == "__main__":
    rng = np.random.default_rng(0)
    x = rng.standard_normal((B, S, D)).astype(np.float32)
    m = (rng.random((B, S, D)) < 0.5).astype(np.float32) * 2.0
    out = kernel(x=x, drop_mask=m)
    print(out.shape, out.dtype)


# revision 15
# speedup vs baseline: 1.3685x; 1.0100x over previous
"""Trainium2 Bass kernel for nn_Attention1 — v4.

Per batch b: out[b] = w @ x with w[k] = sum_q softmax(q x^T/16)[q, k]
(query-summed attention; only softmax column sums are needed).
Data parallel: one batch element per NeuronCore.

Structure:
  Phase A (stage-batched, x-path first so scores can start early):
    x: load f32 -> fp16 cast (DVE 2x) -> DRAM bounce -> XBAR transpose
       -> fp8 cast (GpSimd)  => xT8 [d%128, d//128, s]
    q: mask load; q16 = fp16(x*mask) on GpSimd; diag via q16^2 (DVE TT 2x +
       reduce); exp biases; bounce/transpose/fp8 cast as for x.
  Phase B (32 query stripes of 128):
    scores: fp8 DoubleRow matmuls, K=256 in one pass, N=512 per bank.
    exp: ScalarE activation (bias/scale fused, accum_out row sums) on part
    of each stripe; VectorE fast exp2 (int-bit trick, one fused
    tensor_scalar) + convert/accumulate pass on the rest.
    column sums: M=1 matmuls with r_q = fp16(1/Z_q) stationary, accumulated
    in 2 persistent PSUM banks across all 32 stripes (4 col-groups/bank).
  Tail: evacuate w, transpose into partitions with K=1 matmuls,
    out = w @ x in fp16, DMA out.

Known stack pitfalls honored here: tensor_tensor_reduce aborts at runtime
(use TT+reduce or activation accum); SBUF<->SBUF DMA deadlocks against
XBAR transposes (casts are compute-engine copies); CCE mult-during-DMA
unsupported.
"""

import os
import sys

import numpy as np

_TRN_REPO = "/opt/trn_rl_repo"
if os.path.isdir(_TRN_REPO) and _TRN_REPO not in sys.path:
    sys.path.insert(0, _TRN_REPO)

import concourse.bass as bass
import concourse.mybir as mybir
import concourse.tile as tile
from concourse import bacc
from concourse.bass_utils import run_bass_kernel_spmd

F32 = mybir.dt.float32
F16 = mybir.dt.float16
FP8 = mybir.dt.float8e4
I32 = mybir.dt.int32
OP = mybir.AluOpType
DR = mybir.MatmulPerfMode.DoubleRow

B = 8
S = 4096
D = 256
P = 128

NBLK = S // 512
NST = S // P

E_SHIFT = float(8 * np.log(2.0))
SCALE = 1.0 / 16.0

C_MAGIC = 361007.0
K2 = float((2.0**23) * np.log2(np.e) / 16.0)
CB = float((127.0 * 2.0**23 - C_MAGIC) * 16.0 / ((2.0**23) * np.log2(np.e)))

# per-stripe k tiles and their exp consumers: (k0, kn, [(who, c0, c1), ...])
# "A" = ScalarE activation exp, "V" = VectorE fast exp. Ranges relative to
# tile start. zi = index of the per-stripe row-sum partial each consumer owns.
K_TILES = [
    (0, 1536, [("A", 0, 1536, 0)]),
    (1536, 1024, [("V", 0, 1024, 2)]),
    (2560, 1536, [("A", 0, 1536, 1)]),
]
NZ = 3


def build_kernel(finalize: bool = True) -> bass.Bass:
    nc = bacc.Bacc(None)

    x_in = nc.declare_dram_parameter("x", [S, D], F32, isOutput=False)
    m_in = nc.declare_dram_parameter("mask", [S, D], F32, isOutput=False)
    out_ext = nc.declare_dram_parameter("out", [1, D], F32, isOutput=True)

    x_in_t = x_in.rearrange("(a p) d -> p a d", p=P)
    m_in_t = m_in.rearrange("(a p) d -> p a d", p=P)

    with tile.TileContext(nc) as tc:
        with (
            tc.tile_pool(name="dram", bufs=1, space="DRAM") as dramp,
            tc.tile_pool(name="resident", bufs=1) as res,
            tc.tile_pool(name="qstage", bufs=3) as qstage,
            tc.tile_pool(name="tstage", bufs=4) as tstage,
            tc.tile_pool(name="etile", bufs=8) as ep,
            tc.tile_pool(name="e32", bufs=2) as e32p,
            tc.tile_pool(name="ps_scores", bufs=2, space="PSUM") as pss,
            tc.tile_pool(name="ps_w", bufs=2, space="PSUM") as psw,
        ):
            q16d = dramp.tile([S, D], F16)
            x16d = dramp.tile([S, D], F16)

            xf = res.tile([P, NST, D], F32)
            mkf = res.tile([P, NST, D], F16)
            x16 = res.tile([P, NST, D], F16)
            qT8 = res.tile([P, 2, S], FP8)
            xT8 = res.tile([P, 2, S], FP8)
            bias_all = res.tile([P, NST], F32)
            bias2_all = res.tile([P, NST], F32)
            d2 = res.tile([P, NST], F32)
            zP = [res.tile([P, NST], F32, name=f"zp{i}") for i in range(NZ)]
            zsum = res.tile([P, NST], F32)
            rr = res.tile([P, NST], F32)
            r16 = res.tile([P, NST], F16)
            w16 = res.tile([1, S], F16)
            wtot_sb = res.tile([P, NST], F16)
            ones16 = res.tile([1, 1], F16)
            out_sb = res.tile([1, D], F32)

            nc.vector.memset(ones16[:], 1.0)

            wb = [
                psw.tile([P, 512], F32, tag="w", name=f"wb{i}") for i in range(2)
            ]

            # ---- Phase A ----
            xT16 = res.tile([P, 2, S], F16)

            # x-path: big DMAs, coarse transposes, casts on ScalarE
            nc.scalar.dma_start(xf[:], x_in_t[:])
            nc.vector.tensor_copy(x16[:], xf[:])
            nc.scalar.dma_start(
                x16d.rearrange("(a p) d -> p a d", p=P)[:], x16[:]
            )
            for dh in range(2):
                nc.sync.dma_start(
                    xT16[:, dh, :], x16d[:, dh * P : (dh + 1) * P],
                    transpose=True,
                )
                nc.scalar.copy(xT8[:, dh, :], xT16[:, dh, :])

            # q-path, per block (pipelines under phase B)
            # f32 -> fp16 cast during load ({0,2} exact in fp16)
            nc.gpsimd.dma_start(mkf[:], m_in_t[:])
            for blk in range(NBLK):
                a0 = blk * 4
                rows = slice(blk * 512, (blk + 1) * 512)
                q16 = qstage.tile([P, 4, D], F16, tag="q16")
                nc.gpsimd.tensor_tensor(
                    q16[:], xf[:, a0 : a0 + 4, :], mkf[:, a0 : a0 + 4, :], OP.mult
                )
                # d2 = sum_d q16^2 = 2*diag (mask^2 = 2*mask)
                q2 = qstage.tile([P, 4, D], F16, tag="q2")
                nc.vector.tensor_tensor(q2[:], q16[:], q16[:], OP.mult)
                nc.vector.tensor_reduce(
                    d2[:, a0 : a0 + 4], q2[:], mybir.AxisListType.X, OP.add
                )
                nc.vector.tensor_scalar(
                    bias_all[:, a0 : a0 + 4], d2[:, a0 : a0 + 4],
                    -1.0 / 32.0, E_SHIFT, OP.mult, OP.add,
                )
                nc.vector.tensor_scalar(
                    bias2_all[:, a0 : a0 + 4], d2[:, a0 : a0 + 4],
                    -0.5, 16.0 * E_SHIFT + CB, OP.mult, OP.add,
                )
                nc.sync.dma_start(
                    q16d.rearrange("(a p) d -> p a d", p=P)[:, a0 : a0 + 4, :],
                    q16[:],
                )
                qT16b = tstage.tile([P, 2, 512], F16, tag="qT16b")
                for dh in range(2):
                    nc.sync.dma_start(
                        qT16b[:, dh, :],
                        q16d[rows, dh * P : (dh + 1) * P],
                        transpose=True,
                    )
                nc.scalar.copy(qT8[:, :, rows], qT16b[:])

            # ---- Phase B ----
            def emit_colsum(g, ets):
                for j in range(4):
                    qs = 4 * g + j
                    for c in range(8):
                        nc.tensor.matmul(
                            wb[c // 4][32 * (c % 4) : 32 * (c % 4) + 1, :],
                            lhsT=r16[:, qs : qs + 1],
                            rhs=ets[j][:, c * 512 : (c + 1) * 512],
                            start=(qs == 0),
                            stop=(qs == 31),
                            tile_position=(0, 32 * (c % 4)),
                            skip_group_check=True,
                        )

            prev = None
            for g in range(8):
                ets = []
                for j in range(4):
                    qs = 4 * g + j
                    et = ep.tile([P, S], F16, tag="e")
                    ets.append(et)
                    for k0, kn, consumers in K_TILES:
                        ps = pss.tile([P, 1536], F32, tag="s")
                        for n in range(kn // 512):
                            nc.tensor.matmul(
                                ps[:, n * 512 : (n + 1) * 512],
                                lhsT=qT8[:, :, qs * P : (qs + 1) * P],
                                rhs=xT8[:, :, k0 + n * 512 : k0 + (n + 1) * 512],
                                start=True,
                                stop=True,
                                perf_mode=DR,
                            )
                        for who, c0, c1, zi in consumers:
                            if who == "A":
                                nc.scalar.activation(
                                    out=et[:, k0 + c0 : k0 + c1],
                                    in_=ps[:, c0:c1],
                                    func=mybir.ActivationFunctionType.Exp,
                                    bias=bias_all[:, qs : qs + 1],
                                    scale=SCALE,
                                    accum_out=zP[zi][:, qs : qs + 1],
                                )
                            else:
                                e32 = e32p.tile([P, 1536], F32, tag="e32")
                                nc.vector.tensor_scalar(
                                    e32.bitcast(I32)[:, : c1 - c0],
                                    ps[:, c0:c1],
                                    bias2_all[:, qs : qs + 1],
                                    K2,
                                    OP.add,
                                    OP.mult,
                                )
                                nc.vector.tensor_scalar(
                                    et[:, k0 + c0 : k0 + c1],
                                    e32[:, : c1 - c0],
                                    1.0,
                                    None,
                                    OP.mult,
                                    OP.add,
                                    accum_out=zP[zi][:, qs : qs + 1],
                                )
                    if j == 0 and prev is not None:
                        emit_colsum(g - 1, prev)
                        prev = None
                sl = slice(4 * g, 4 * g + 4)
                nc.vector.tensor_tensor(zsum[:, sl], zP[0][:, sl], zP[1][:, sl], OP.add)
                nc.vector.tensor_tensor(zsum[:, sl], zsum[:, sl], zP[2][:, sl], OP.add)
                nc.vector.reciprocal(rr[:, sl], zsum[:, sl])
                nc.vector.tensor_copy(r16[:, sl], rr[:, sl])
                prev = ets
            emit_colsum(7, prev)

            # ---- Tail ----
            for c in range(8):
                src = wb[c // 4][32 * (c % 4) : 32 * (c % 4) + 1, :]
                dst = w16[:, c * 512 : (c + 1) * 512]
                if c % 2 == 0:
                    nc.vector.tensor_copy(dst, src)
                else:
                    nc.scalar.copy(dst, src)

            wtotP = psw.tile([P, NST], F32, tag="w")
            for cc in range(NST):
                nc.tensor.matmul(
                    wtotP[:, cc : cc + 1],
                    lhsT=w16[:, cc * P : (cc + 1) * P],
                    rhs=ones16[:],
                    start=True,
                    stop=True,
                )
            nc.vector.tensor_copy(wtot_sb[:], wtotP[:])

            po = psw.tile([1, D], F32, tag="w")
            for cc in range(NST):
                nc.tensor.matmul(
                    po[:],
                    lhsT=wtot_sb[:, cc : cc + 1],
                    rhs=x16[:, cc, :],
                    start=(cc == 0),
                    stop=(cc == NST - 1),
                )
            nc.scalar.copy(out_sb[:], po[:])
            nc.sync.dma_start(out_ext[:, :], out_sb[:])

    if finalize:
        nc.finalize()
    return nc


def _run(x: np.ndarray, drop_mask: np.ndarray, trace: bool = False, nc=None):
    if nc is None:
        nc = build_kernel()
    in_maps = [{"x": x[b], "mask": drop_mask[b]} for b in range(B)]
    res = run_bass_kernel_spmd(nc, in_maps, list(range(B)), trace=trace)
    out = np.stack([res.results[b]["out"].reshape(D) for b in range(B)])
    return out.astype(np.float32), res


def kernel(**inputs: np.ndarray) -> np.ndarray:
    x = np.ascontiguousarray(inputs["x"], dtype=np.float32)
    drop_mask = np.ascontiguousarray(inputs["drop_mask"], dtype=np.float32)
    assert x.shape == (B, S, D) and drop_mask.shape == (B, S, D)
    out, _ = _run(x, drop_mask)
    return out


def profile(**inputs: np.ndarray):
    x = np.ascontiguousarray(inputs["x"], dtype=np.float32)
    drop_mask = np.ascontiguousarray(inputs["drop_mask"], dtype=np.float32)
    out, res = _run(x, drop_mask, trace=True)
    return res.exec_time_ns


if __name__ == "__main__":
    rng = np.random.default_rng(0)
    x = rng.standard_normal((B, S, D)).astype(np.float32)
    m = (rng.random((B, S, D)) < 0.5).astype(np.float32) * 2.0
    out = kernel(x=x, drop_mask=m)
    print(out.shape, out.dtype)


# revision 16
# speedup vs baseline: 1.3997x; 1.0228x over previous
"""Trainium2 Bass kernel for nn_Attention1 — v4.

Per batch b: out[b] = w @ x with w[k] = sum_q softmax(q x^T/16)[q, k]
(query-summed attention; only softmax column sums are needed).
Data parallel: one batch element per NeuronCore.

Structure:
  Phase A (stage-batched, x-path first so scores can start early):
    x: load f32 -> fp16 cast (DVE 2x) -> DRAM bounce -> XBAR transpose
       -> fp8 cast (GpSimd)  => xT8 [d%128, d//128, s]
    q: mask load; q16 = fp16(x*mask) on GpSimd; diag via q16^2 (DVE TT 2x +
       reduce); exp biases; bounce/transpose/fp8 cast as for x.
  Phase B (32 query stripes of 128):
    scores: fp8 DoubleRow matmuls, K=256 in one pass, N=512 per bank.
    exp: ScalarE activation (bias/scale fused, accum_out row sums) on part
    of each stripe; VectorE fast exp2 (int-bit trick, one fused
    tensor_scalar) + convert/accumulate pass on the rest.
    column sums: M=1 matmuls with r_q = fp16(1/Z_q) stationary, accumulated
    in 2 persistent PSUM banks across all 32 stripes (4 col-groups/bank).
  Tail: evacuate w, transpose into partitions with K=1 matmuls,
    out = w @ x in fp16, DMA out.

Known stack pitfalls honored here: tensor_tensor_reduce aborts at runtime
(use TT+reduce or activation accum); SBUF<->SBUF DMA deadlocks against
XBAR transposes (casts are compute-engine copies); CCE mult-during-DMA
unsupported.
"""

import os
import sys

import numpy as np

_TRN_REPO = "/opt/trn_rl_repo"
if os.path.isdir(_TRN_REPO) and _TRN_REPO not in sys.path:
    sys.path.insert(0, _TRN_REPO)

import concourse.bass as bass
import concourse.mybir as mybir
import concourse.tile as tile
from concourse import bacc
from concourse.bass_utils import run_bass_kernel_spmd

F32 = mybir.dt.float32
F16 = mybir.dt.float16
FP8 = mybir.dt.float8e4
I32 = mybir.dt.int32
OP = mybir.AluOpType
DR = mybir.MatmulPerfMode.DoubleRow

B = 8
S = 4096
D = 256
P = 128

NBLK = S // 512
NST = S // P

E_SHIFT = float(8 * np.log(2.0))
SCALE = 1.0 / 16.0

C_MAGIC = 361007.0
K2 = float((2.0**23) * np.log2(np.e) / 16.0)
CB = float((127.0 * 2.0**23 - C_MAGIC) * 16.0 / ((2.0**23) * np.log2(np.e)))

# per-stripe k tiles and their exp consumers: (k0, kn, [(who, c0, c1), ...])
# "A" = ScalarE activation exp, "V" = VectorE fast exp. Ranges relative to
# tile start. zi = index of the per-stripe row-sum partial each consumer owns.
K_TILES = [
    (0, 1536, [("A", 0, 1536, 0)]),
    (1536, 1024, [("V", 0, 1024, 2)]),
    (2560, 1536, [("A", 0, 1536, 1)]),
]
NZ = 3


def build_kernel(finalize: bool = True) -> bass.Bass:
    nc = bacc.Bacc(None)

    x_in = nc.declare_dram_parameter("x", [S, D], F32, isOutput=False)
    m_in = nc.declare_dram_parameter("mask", [S, D], F32, isOutput=False)
    out_ext = nc.declare_dram_parameter("out", [1, D], F32, isOutput=True)

    x_in_t = x_in.rearrange("(a p) d -> p a d", p=P)
    m_in_t = m_in.rearrange("(a p) d -> p a d", p=P)

    with tile.TileContext(nc) as tc:
        with (
            tc.tile_pool(name="dram", bufs=1, space="DRAM") as dramp,
            tc.tile_pool(name="resident", bufs=1) as res,
            tc.tile_pool(name="qstage", bufs=3) as qstage,
            tc.tile_pool(name="tstage", bufs=4) as tstage,
            tc.tile_pool(name="etile", bufs=8) as ep,
            tc.tile_pool(name="e32", bufs=2) as e32p,
            tc.tile_pool(name="ps_scores", bufs=2, space="PSUM") as pss,
            tc.tile_pool(name="ps_w", bufs=2, space="PSUM") as psw,
        ):
            q16d = dramp.tile([S, D], F16)
            x16d = dramp.tile([S, D], F16)

            xf = res.tile([P, NST, D], F32)
            mkf = res.tile([P, NST, D], F16)
            x16 = res.tile([P, NST, D], F16)
            qT8 = res.tile([P, 2, S], FP8)
            xT8 = res.tile([P, 2, S], FP8)
            bias_all = res.tile([P, NST], F32)
            bias2_all = res.tile([P, NST], F32)
            d2 = res.tile([P, NST], F32)
            zP = [res.tile([P, NST], F32, name=f"zp{i}") for i in range(NZ)]
            zsum = res.tile([P, NST], F32)
            rr = res.tile([P, NST], F32)
            r16 = res.tile([P, NST], F16)
            w16 = res.tile([1, S], F16)
            wtot_sb = res.tile([P, NST], F16)
            ones16 = res.tile([1, 1], F16)
            out_sb = res.tile([1, D], F32)

            nc.vector.memset(ones16[:], 1.0)

            wb = [
                psw.tile([P, 512], F32, tag="w", name=f"wb{i}") for i in range(2)
            ]

            # ---- Phase A ----
            # x-path: 4 row-chunks of 1024, stage-batched so load/cast/
            # bounce/transpose/fp8-cast pipeline and phase B can start on
            # the first k-chunks
            NXC = 4
            XC = S // NXC                     # 1024 rows per chunk
            XCA = XC // P                     # 8 stripe-rows per chunk
            for c in range(NXC):
                a0 = c * XCA
                eng = nc.scalar if c % 2 == 0 else nc.sync
                eng.dma_start(
                    xf[:, a0 : a0 + XCA, :], x_in_t[:, a0 : a0 + XCA, :]
                )
            for c in range(NXC):
                a0 = c * XCA
                nc.vector.tensor_copy(
                    x16[:, a0 : a0 + XCA, :], xf[:, a0 : a0 + XCA, :]
                )
            for c in range(NXC):
                a0 = c * XCA
                eng = nc.scalar if c % 2 == 0 else nc.sync
                eng.dma_start(
                    x16d.rearrange("(a p) d -> p a d", p=P)[:, a0 : a0 + XCA, :],
                    x16[:, a0 : a0 + XCA, :],
                )
            for c in range(NXC):
                rows = slice(c * XC, (c + 1) * XC)
                xT16c = tstage.tile([P, 2, XC], F16, tag="xT16c")
                for dh in range(2):
                    nc.sync.dma_start(
                        xT16c[:, dh, :],
                        x16d[rows, dh * P : (dh + 1) * P],
                        transpose=True,
                    )
                nc.scalar.copy(xT8[:, :, rows], xT16c[:])

            # q-path, per block (pipelines under phase B)
            # f32 -> fp16 cast during load ({0,2} exact in fp16)
            nc.gpsimd.dma_start(mkf[:], m_in_t[:])
            for blk in range(NBLK):
                a0 = blk * 4
                rows = slice(blk * 512, (blk + 1) * 512)
                q16 = qstage.tile([P, 4, D], F16, tag="q16")
                nc.gpsimd.tensor_tensor(
                    q16[:], xf[:, a0 : a0 + 4, :], mkf[:, a0 : a0 + 4, :], OP.mult
                )
                # d2 = sum_d q16^2 = 2*diag (mask^2 = 2*mask)
                q2 = qstage.tile([P, 4, D], F16, tag="q2")
                nc.vector.tensor_tensor(q2[:], q16[:], q16[:], OP.mult)
                nc.vector.tensor_reduce(
                    d2[:, a0 : a0 + 4], q2[:], mybir.AxisListType.X, OP.add
                )
                nc.vector.tensor_scalar(
                    bias_all[:, a0 : a0 + 4], d2[:, a0 : a0 + 4],
                    -1.0 / 32.0, E_SHIFT, OP.mult, OP.add,
                )
                nc.vector.tensor_scalar(
                    bias2_all[:, a0 : a0 + 4], d2[:, a0 : a0 + 4],
                    -0.5, 16.0 * E_SHIFT + CB, OP.mult, OP.add,
                )
                nc.sync.dma_start(
                    q16d.rearrange("(a p) d -> p a d", p=P)[:, a0 : a0 + 4, :],
                    q16[:],
                )
                qT16b = tstage.tile([P, 2, 512], F16, tag="qT16b")
                for dh in range(2):
                    nc.sync.dma_start(
                        qT16b[:, dh, :],
                        q16d[rows, dh * P : (dh + 1) * P],
                        transpose=True,
                    )
                nc.gpsimd.tensor_copy(qT8[:, :, rows], qT16b[:])

            # ---- Phase B ----
            # HAM warm-up: ~20 dense DR matmuls so the PE clock ramps to
            # 2.4 GHz before the real score stream begins
            warm = pss.tile([P, 1536], F32, tag="s", name="warm")
            for i in range(20):
                nc.tensor.matmul(
                    warm[:, (i % 3) * 512 : (i % 3) * 512 + 512],
                    lhsT=xT8[:, :, 0:P],
                    rhs=xT8[:, :, 0:512],
                    start=True,
                    stop=True,
                    perf_mode=DR,
                )

            def emit_colsum(g, ets):
                for j in range(4):
                    qs = 4 * g + j
                    for c in range(8):
                        nc.tensor.matmul(
                            wb[c // 4][32 * (c % 4) : 32 * (c % 4) + 1, :],
                            lhsT=r16[:, qs : qs + 1],
                            rhs=ets[j][:, c * 512 : (c + 1) * 512],
                            start=(qs == 0),
                            stop=(qs == 31),
                            tile_position=(0, 32 * (c % 4)),
                            skip_group_check=True,
                        )

            prev = None
            for g in range(8):
                ets = []
                for j in range(4):
                    qs = 4 * g + j
                    et = ep.tile([P, S], F16, tag="e")
                    ets.append(et)
                    for k0, kn, consumers in K_TILES:
                        ps = pss.tile([P, 1536], F32, tag="s")
                        for n in range(kn // 512):
                            nc.tensor.matmul(
                                ps[:, n * 512 : (n + 1) * 512],
                                lhsT=qT8[:, :, qs * P : (qs + 1) * P],
                                rhs=xT8[:, :, k0 + n * 512 : k0 + (n + 1) * 512],
                                start=True,
                                stop=True,
                                perf_mode=DR,
                            )
                        for who, c0, c1, zi in consumers:
                            if who == "A":
                                nc.scalar.activation(
                                    out=et[:, k0 + c0 : k0 + c1],
                                    in_=ps[:, c0:c1],
                                    func=mybir.ActivationFunctionType.Exp,
                                    bias=bias_all[:, qs : qs + 1],
                                    scale=SCALE,
                                    accum_out=zP[zi][:, qs : qs + 1],
                                )
                            else:
                                e32 = e32p.tile([P, 1536], F32, tag="e32")
                                nc.vector.tensor_scalar(
                                    e32.bitcast(I32)[:, : c1 - c0],
                                    ps[:, c0:c1],
                                    bias2_all[:, qs : qs + 1],
                                    K2,
                                    OP.add,
                                    OP.mult,
                                )
                                nc.vector.tensor_scalar(
                                    et[:, k0 + c0 : k0 + c1],
                                    e32[:, : c1 - c0],
                                    1.0,
                                    None,
                                    OP.mult,
                                    OP.add,
                                    accum_out=zP[zi][:, qs : qs + 1],
                                )
                    if j == 0 and prev is not None:
                        emit_colsum(g - 1, prev)
                        prev = None
                sl = slice(4 * g, 4 * g + 4)
                nc.vector.tensor_tensor(zsum[:, sl], zP[0][:, sl], zP[1][:, sl], OP.add)
                nc.vector.tensor_tensor(zsum[:, sl], zsum[:, sl], zP[2][:, sl], OP.add)
                nc.vector.reciprocal(rr[:, sl], zsum[:, sl])
                nc.vector.tensor_copy(r16[:, sl], rr[:, sl])
                prev = ets
            emit_colsum(7, prev)

            # ---- Tail ----
            for c in range(8):
                src = wb[c // 4][32 * (c % 4) : 32 * (c % 4) + 1, :]
                dst = w16[:, c * 512 : (c + 1) * 512]
                if c % 2 == 0:
                    nc.vector.tensor_copy(dst, src)
                else:
                    nc.scalar.copy(dst, src)

            wtotP = psw.tile([P, NST], F32, tag="w")
            for cc in range(NST):
                nc.tensor.matmul(
                    wtotP[:, cc : cc + 1],
                    lhsT=w16[:, cc * P : (cc + 1) * P],
                    rhs=ones16[:],
                    start=True,
                    stop=True,
                )
            nc.vector.tensor_copy(wtot_sb[:], wtotP[:])

            po = psw.tile([1, D], F32, tag="w")
            for cc in range(NST):
                nc.tensor.matmul(
                    po[:],
                    lhsT=wtot_sb[:, cc : cc + 1],
                    rhs=x16[:, cc, :],
                    start=(cc == 0),
                    stop=(cc == NST - 1),
                )
            nc.scalar.copy(out_sb[:], po[:])
            nc.sync.dma_start(out_ext[:, :], out_sb[:])

    if finalize:
        nc.finalize()
    return nc


def _run(x: np.ndarray, drop_mask: np.ndarray, trace: bool = False, nc=None):
    if nc is None:
        nc = build_kernel()
    in_maps = [{"x": x[b], "mask": drop_mask[b]} for b in range(B)]
    res = run_bass_kernel_spmd(nc, in_maps, list(range(B)), trace=trace)
    out = np.stack([res.results[b]["out"].reshape(D) for b in range(B)])
    return out.astype(np.float32), res


def kernel(**inputs: np.ndarray) -> np.ndarray:
    x = np.ascontiguousarray(inputs["x"], dtype=np.float32)
    drop_mask = np.ascontiguousarray(inputs["drop_mask"], dtype=np.float32)
    assert x.shape == (B, S, D) and drop_mask.shape == (B, S, D)
    out, _ = _run(x, drop_mask)
    return out


def profile(**inputs: np.ndarray):
    x = np.ascontiguousarray(inputs["x"], dtype=np.float32)
    drop_mask = np.ascontiguousarray(inputs["drop_mask"], dtype=np.float32)
    out, res = _run(x, drop_mask, trace=True)
    return res.exec_time_ns


if __name__ == "__main__":
    rng = np.random.default_rng(0)
    x = rng.standard_normal((B, S, D)).astype(np.float32)
    m = (rng.random((B, S, D)) < 0.5).astype(np.float32) * 2.0
    out = kernel(x=x, drop_mask=m)
    print(out.shape, out.dtype)


# revision 17
# speedup vs baseline: 1.4379x; 1.0273x over previous
"""Trainium2 Bass kernel for nn_Attention1 — v4.

Per batch b: out[b] = w @ x with w[k] = sum_q softmax(q x^T/16)[q, k]
(query-summed attention; only softmax column sums are needed).
Data parallel: one batch element per NeuronCore.

Structure:
  Phase A (stage-batched, x-path first so scores can start early):
    x: load f32 -> fp16 cast (DVE 2x) -> DRAM bounce -> XBAR transpose
       -> fp8 cast (GpSimd)  => xT8 [d%128, d//128, s]
    q: mask load; q16 = fp16(x*mask) on GpSimd; diag via q16^2 (DVE TT 2x +
       reduce); exp biases; bounce/transpose/fp8 cast as for x.
  Phase B (32 query stripes of 128):
    scores: fp8 DoubleRow matmuls, K=256 in one pass, N=512 per bank.
    exp: ScalarE activation (bias/scale fused, accum_out row sums) on part
    of each stripe; VectorE fast exp2 (int-bit trick, one fused
    tensor_scalar) + convert/accumulate pass on the rest.
    column sums: M=1 matmuls with r_q = fp16(1/Z_q) stationary, accumulated
    in 2 persistent PSUM banks across all 32 stripes (4 col-groups/bank).
  Tail: evacuate w, transpose into partitions with K=1 matmuls,
    out = w @ x in fp16, DMA out.

Known stack pitfalls honored here: tensor_tensor_reduce aborts at runtime
(use TT+reduce or activation accum); SBUF<->SBUF DMA deadlocks against
XBAR transposes (casts are compute-engine copies); CCE mult-during-DMA
unsupported.
"""

import os
import sys

import numpy as np

_TRN_REPO = "/opt/trn_rl_repo"
if os.path.isdir(_TRN_REPO) and _TRN_REPO not in sys.path:
    sys.path.insert(0, _TRN_REPO)

import concourse.bass as bass
import concourse.mybir as mybir
import concourse.tile as tile
from concourse import bacc
from concourse.bass_utils import run_bass_kernel_spmd

F32 = mybir.dt.float32
F16 = mybir.dt.float16
FP8 = mybir.dt.float8e4
I32 = mybir.dt.int32
OP = mybir.AluOpType
DR = mybir.MatmulPerfMode.DoubleRow

B = 8
S = 4096
D = 256
P = 128

NBLK = S // 512
NST = S // P

E_SHIFT = float(8 * np.log(2.0))
SCALE = 1.0 / 16.0

C_MAGIC = 361007.0
K2 = float((2.0**23) * np.log2(np.e) / 16.0)
CB = float((127.0 * 2.0**23 - C_MAGIC) * 16.0 / ((2.0**23) * np.log2(np.e)))

# per-stripe k tiles and their exp consumers: (k0, kn, [(who, c0, c1), ...])
# "A" = ScalarE activation exp, "V" = VectorE fast exp. Ranges relative to
# tile start. zi = index of the per-stripe row-sum partial each consumer owns.
K_TILES = [
    (0, 1536, [("A", 0, 1536, 0)]),
    (1536, 1024, [("V", 0, 1024, 2)]),
    (2560, 1536, [("A", 0, 1536, 1)]),
]
NZ = 3


def build_kernel(finalize: bool = True) -> bass.Bass:
    nc = bacc.Bacc(None)

    x_in = nc.declare_dram_parameter("x", [S, D], F32, isOutput=False)
    m_in = nc.declare_dram_parameter("mask", [S, D], F32, isOutput=False)
    out_ext = nc.declare_dram_parameter("out", [1, D], F32, isOutput=True)

    x_in_t = x_in.rearrange("(a p) d -> p a d", p=P)
    m_in_t = m_in.rearrange("(a p) d -> p a d", p=P)

    with tile.TileContext(nc) as tc:
        with (
            tc.tile_pool(name="dram", bufs=1, space="DRAM") as dramp,
            tc.tile_pool(name="resident", bufs=1) as res,
            tc.tile_pool(name="qstage", bufs=3) as qstage,
            tc.tile_pool(name="tstage", bufs=4) as tstage,
            tc.tile_pool(name="etile", bufs=8) as ep,
            tc.tile_pool(name="e32", bufs=2) as e32p,
            tc.tile_pool(name="ps_scores", bufs=2, space="PSUM") as pss,
            tc.tile_pool(name="ps_w", bufs=2, space="PSUM") as psw,
        ):
            q16d = dramp.tile([S, D], F16)
            x16d = dramp.tile([S, D], F16)

            xf = res.tile([P, NST, D], F32)
            mkf = res.tile([P, NST, D], F16)
            x16 = res.tile([P, NST, D], F16)
            qT8 = res.tile([P, 2, S], FP8)
            xT8 = res.tile([P, 2, S], FP8)
            bias_all = res.tile([P, NST], F32)
            bias2_all = res.tile([P, NST], F32)
            d2 = res.tile([P, NST], F32)
            zP = [res.tile([P, NST], F32, name=f"zp{i}") for i in range(NZ)]
            zsum = res.tile([P, NST], F32)
            rr = res.tile([P, NST], F32)
            r16 = res.tile([P, NST], F16)
            w16 = res.tile([1, S], F16)
            wtot_sb = res.tile([P, NST], F16)
            ones16 = res.tile([1, 1], F16)
            out_sb = res.tile([1, D], F32)

            nc.vector.memset(ones16[:], 1.0)

            wb = [
                psw.tile([P, 512], F32, tag="w", name=f"wb{i}") for i in range(2)
            ]

            # ---- Phase A ----
            # x-path: 4 row-chunks of 1024, stage-batched so load/cast/
            # bounce/transpose/fp8-cast pipeline and phase B can start on
            # the first k-chunks
            NXC = 4
            XC = S // NXC                     # 1024 rows per chunk
            XCA = XC // P                     # 8 stripe-rows per chunk
            for c in range(NXC):
                a0 = c * XCA
                eng = nc.scalar if c % 2 == 0 else nc.sync
                eng.dma_start(
                    xf[:, a0 : a0 + XCA, :], x_in_t[:, a0 : a0 + XCA, :]
                )
            for c in range(NXC):
                a0 = c * XCA
                nc.vector.tensor_copy(
                    x16[:, a0 : a0 + XCA, :], xf[:, a0 : a0 + XCA, :]
                )
            for c in range(NXC):
                a0 = c * XCA
                eng = nc.scalar if c % 2 == 0 else nc.sync
                eng.dma_start(
                    x16d.rearrange("(a p) d -> p a d", p=P)[:, a0 : a0 + XCA, :],
                    x16[:, a0 : a0 + XCA, :],
                )
            for c in range(NXC):
                rows = slice(c * XC, (c + 1) * XC)
                xT16c = tstage.tile([P, 2, XC], F16, tag="xT16c")
                for dh in range(2):
                    nc.sync.dma_start(
                        xT16c[:, dh, :],
                        x16d[rows, dh * P : (dh + 1) * P],
                        transpose=True,
                    )
                nc.scalar.copy(xT8[:, :, rows], xT16c[:])

            # q-path, per block (pipelines under phase B)
            for blk in range(NBLK):
                a0 = blk * 4
                # f32 -> fp16 cast during load ({0,2} exact in fp16)
                nc.gpsimd.dma_start(
                    mkf[:, a0 : a0 + 4, :], m_in_t[:, a0 : a0 + 4, :]
                )
            for blk in range(NBLK):
                a0 = blk * 4
                rows = slice(blk * 512, (blk + 1) * 512)
                q16 = qstage.tile([P, 4, D], F16, tag="q16")
                nc.gpsimd.tensor_tensor(
                    q16[:], xf[:, a0 : a0 + 4, :], mkf[:, a0 : a0 + 4, :], OP.mult
                )
                # d2 = sum_d q16^2 = 2*diag (mask^2 = 2*mask)
                q2 = qstage.tile([P, 4, D], F16, tag="q2")
                nc.vector.tensor_tensor(q2[:], q16[:], q16[:], OP.mult)
                nc.vector.tensor_reduce(
                    d2[:, a0 : a0 + 4], q2[:], mybir.AxisListType.X, OP.add
                )
                nc.vector.tensor_scalar(
                    bias_all[:, a0 : a0 + 4], d2[:, a0 : a0 + 4],
                    -1.0 / 32.0, E_SHIFT, OP.mult, OP.add,
                )
                nc.vector.tensor_scalar(
                    bias2_all[:, a0 : a0 + 4], d2[:, a0 : a0 + 4],
                    -0.5, 16.0 * E_SHIFT + CB, OP.mult, OP.add,
                )
                nc.sync.dma_start(
                    q16d.rearrange("(a p) d -> p a d", p=P)[:, a0 : a0 + 4, :],
                    q16[:],
                )
                qT16b = tstage.tile([P, 2, 512], F16, tag="qT16b")
                for dh in range(2):
                    nc.sync.dma_start(
                        qT16b[:, dh, :],
                        q16d[rows, dh * P : (dh + 1) * P],
                        transpose=True,
                    )
                nc.gpsimd.tensor_copy(qT8[:, :, rows], qT16b[:])

            # ---- Phase B ----
            # HAM warm-up: ~20 dense DR matmuls so the PE clock ramps to
            # 2.4 GHz before the real score stream begins
            warm = pss.tile([P, 1536], F32, tag="s", name="warm")
            for i in range(20):
                nc.tensor.matmul(
                    warm[:, (i % 3) * 512 : (i % 3) * 512 + 512],
                    lhsT=xT8[:, :, 0:P],
                    rhs=xT8[:, :, 0:512],
                    start=True,
                    stop=True,
                    perf_mode=DR,
                )

            def emit_colsum(g, ets):
                for j in range(4):
                    qs = 4 * g + j
                    for c in range(8):
                        nc.tensor.matmul(
                            wb[c // 4][32 * (c % 4) : 32 * (c % 4) + 1, :],
                            lhsT=r16[:, qs : qs + 1],
                            rhs=ets[j][:, c * 512 : (c + 1) * 512],
                            start=(qs == 0),
                            stop=(qs == 31),
                            tile_position=(0, 32 * (c % 4)),
                            skip_group_check=True,
                        )

            prev = None
            for g in range(8):
                ets = []
                for j in range(4):
                    qs = 4 * g + j
                    et = ep.tile([P, S], F16, tag="e")
                    ets.append(et)
                    for k0, kn, consumers in K_TILES:
                        ps = pss.tile([P, 1536], F32, tag="s")
                        for n in range(kn // 512):
                            nc.tensor.matmul(
                                ps[:, n * 512 : (n + 1) * 512],
                                lhsT=qT8[:, :, qs * P : (qs + 1) * P],
                                rhs=xT8[:, :, k0 + n * 512 : k0 + (n + 1) * 512],
                                start=True,
                                stop=True,
                                perf_mode=DR,
                            )
                        for who, c0, c1, zi in consumers:
                            if who == "A":
                                nc.scalar.activation(
                                    out=et[:, k0 + c0 : k0 + c1],
                                    in_=ps[:, c0:c1],
                                    func=mybir.ActivationFunctionType.Exp,
                                    bias=bias_all[:, qs : qs + 1],
                                    scale=SCALE,
                                    accum_out=zP[zi][:, qs : qs + 1],
                                )
                            else:
                                e32 = e32p.tile([P, 1536], F32, tag="e32")
                                nc.vector.tensor_scalar(
                                    e32.bitcast(I32)[:, : c1 - c0],
                                    ps[:, c0:c1],
                                    bias2_all[:, qs : qs + 1],
                                    K2,
                                    OP.add,
                                    OP.mult,
                                )
                                nc.vector.tensor_scalar(
                                    et[:, k0 + c0 : k0 + c1],
                                    e32[:, : c1 - c0],
                                    1.0,
                                    None,
                                    OP.mult,
                                    OP.add,
                                    accum_out=zP[zi][:, qs : qs + 1],
                                )
                    if j == 0 and prev is not None:
                        emit_colsum(g - 1, prev)
                        prev = None
                sl = slice(4 * g, 4 * g + 4)
                nc.vector.tensor_tensor(zsum[:, sl], zP[0][:, sl], zP[1][:, sl], OP.add)
                nc.vector.tensor_tensor(zsum[:, sl], zsum[:, sl], zP[2][:, sl], OP.add)
                nc.vector.reciprocal(rr[:, sl], zsum[:, sl])
                nc.vector.tensor_copy(r16[:, sl], rr[:, sl])
                prev = ets
            emit_colsum(7, prev)

            # ---- Tail ----
            for c in range(8):
                src = wb[c // 4][32 * (c % 4) : 32 * (c % 4) + 1, :]
                dst = w16[:, c * 512 : (c + 1) * 512]
                if c % 2 == 0:
                    nc.vector.tensor_copy(dst, src)
                else:
                    nc.scalar.copy(dst, src)

            wtotP = psw.tile([P, NST], F32, tag="w")
            for cc in range(NST):
                nc.tensor.matmul(
                    wtotP[:, cc : cc + 1],
                    lhsT=w16[:, cc * P : (cc + 1) * P],
                    rhs=ones16[:],
                    start=True,
                    stop=True,
                )
            nc.vector.tensor_copy(wtot_sb[:], wtotP[:])

            po = psw.tile([1, D], F32, tag="w")
            for cc in range(NST):
                nc.tensor.matmul(
                    po[:],
                    lhsT=wtot_sb[:, cc : cc + 1],
                    rhs=x16[:, cc, :],
                    start=(cc == 0),
                    stop=(cc == NST - 1),
                )
            nc.scalar.copy(out_sb[:], po[:])
            nc.sync.dma_start(out_ext[:, :], out_sb[:])

    if finalize:
        nc.finalize()
    return nc


def _run(x: np.ndarray, drop_mask: np.ndarray, trace: bool = False, nc=None):
    if nc is None:
        nc = build_kernel()
    in_maps = [{"x": x[b], "mask": drop_mask[b]} for b in range(B)]
    res = run_bass_kernel_spmd(nc, in_maps, list(range(B)), trace=trace)
    out = np.stack([res.results[b]["out"].reshape(D) for b in range(B)])
    return out.astype(np.float32), res


def kernel(**inputs: np.ndarray) -> np.ndarray:
    x = np.ascontiguousarray(inputs["x"], dtype=np.float32)
    drop_mask = np.ascontiguousarray(inputs["drop_mask"], dtype=np.float32)
    assert x.shape == (B, S, D) and drop_mask.shape == (B, S, D)
    out, _ = _run(x, drop_mask)
    return out


def profile(**inputs: np.ndarray):
    x = np.ascontiguousarray(inputs["x"], dtype=np.float32)
    drop_mask = np.ascontiguousarray(inputs["drop_mask"], dtype=np.float32)
    out, res = _run(x, drop_mask, trace=True)
    return res.exec_time_ns


if __name__ == "__main__":
    rng = np.random.default_rng(0)
    x = rng.standard_normal((B, S, D)).astype(np.float32)
    m = (rng.random((B, S, D)) < 0.5).astype(np.float32) * 2.0
    out = kernel(x=x, drop_mask=m)
    print(out.shape, out.dtype)
